# revision 8
# baseline (speedup 1.0000x reference)
"""Trainium2 Bass kernel for nn_LinearLayer_45243185496808.

Computes out[b,o] = sum_i tanh(x[b,i]*t) * (sum_p coef[o,i,p]) with
B=131072, I=O=128, P_NUM=16, data-parallel over batch on 8 NeuronCores.

The kernel is elementwise-walled: ScalarE tanh (1x @1.2GHz) plus the
PSUM->SBUF fp32 eviction (1x on DVE @0.96GHz / ScalarE) are ~16-17us of
combined engine time that nothing else can absorb (GpSimd has no PSUM
port, DMA can't read PSUM, TRN2 matmul output must be fp32). Schedule
highlights, all trace-derived:

  - x ships as fp8e4 [i=128, b] (pure transpose, 1B/elem): 2 MiB/core,
    ACTIVATE reads fp8 at the same 1x rate. End-to-end rel err ~1.5e-2
    vs the 2e-2 gate (dominated by fp8 quantization of x).
  - transposed output: wT [i,o] f16 is the PE stationary; tanh values
    [i,b] f16 stream as the moving operand in N=512 blocks. PSUM holds
    out.T; host transposes the stored [128, B] f16 back.
  - HAM clock: the PE runs N=512 matmuls at 215ns only in the k=8/8
    activity state and ~630ns at k=4 (measured). Warmup matmuls on a
    zero scratch run from kernel start until the coef DMA lands so the
    PE hits k=8 before the w-reduction, and trailing dummy matmuls keep
    it there through the store drain + the framework's fixed per-engine
    semaphore-reset storm (whose Tensor-side resets run 2x faster at
    k=8 — the storm is ~half the measured tail).
  - loads alternate sync/gpsimd rings so chunk sems arrive at ~0.6us
    spacing despite the ~2.5-3us per-DMA completion latency; the coef
    (with an identity block for the w = sum_p coef reduction via 16
    accumulating identity matmuls) is the first issue on each ring.
  - PSUM = one [1536] x2 ring (6 banks) for the main chunks + one
    [512] x2 ring (2 banks) shared by warmup/w/dummies and the tail
    chunks, so the tail's matmuls never queue behind DVE's eviction
    backlog on the big ring.
  - eviction split: DVE takes ~14.3k cols; ScalarE takes the final 2048
    (c5 tail + c6 + c7) emitted after the last tanh, so both engines
    drain together; stores ride gpsimd/SWDGE except the last chunks on
    the by-then-idle sync/HWDGE ring.

HBM per core: 2 MiB x(fp8) + 0.53 MiB coef+identity + 4 MiB out(f16).
"""

import os
import sys
import types

import ml_dtypes
import numpy as np

import concourse.bass as bass
import concourse.mybir as mybir
import concourse.tile as tile
from concourse import bacc
from concourse.bass_utils import run_bass_kernel_spmd


def _ensure_ntff_hook():
    """Register the axon NTFF profile hook if the image lacks antenv.axon_hooks.

    Only needed for BASS_TRACE=1 profiling runs; harmless otherwise."""
    if "antenv.axon_hooks" in sys.modules:
        return
    try:
        from antenv.axon_hooks import get_axon_ntff_profile_hook  # noqa: F401

        return  # real module importable
    except ImportError:
        pass
    hook = None
    try:
        from trn_agent_boot.trn_boot import _ntff_profile_via_ctypes

        so_path = "/opt/axon/libaxon_pjrt.so"
        if os.path.exists(so_path):
            hook = _ntff_profile_via_ctypes(so_path)
    except Exception:
        hook = None
    mod = types.ModuleType("antenv.axon_hooks")
    mod.get_axon_ntff_profile_hook = lambda: hook
    mod.set_axon_ntff_profile_hook = lambda h: None
    sys.modules["antenv.axon_hooks"] = mod


N_CORES = 8
B_FULL = 131072
I_DIM = 128
O_DIM = 128
P_NUM = 16
P = 128                     # SBUF partitions
B_CORE = B_FULL // N_CORES  # 16384

# Load chunks of xt == tanh tiles. Fine taper up front (load-completion
# sems lag ~5us behind issue once several DMAs queue on a ring, so the
# early chunks must be small to arrive in tanh order), big middle
# (amortize the 352-cycle ACTIVATE overhead), taper down (short drain).
WIDTHS = [512, 1024, 1536, 2560, 4096, 3584, 1536, 1024, 512]
assert sum(WIDTHS) == B_CORE
CHUNKS = []
_b = 0
for _w in WIDTHS:
    CHUNKS.append((_b, _w))
    _b += _w
N_TAIL = 2                  # c7, c8: tail PSUM ring + ScalarE eviction

NMM = 512                   # moving cols per matmul = one PSUM bank of f32
MAIN_T = 1536               # main PSUM tile cols (3 banks), x2 bufs
TAIL_T = 512                # tail PSUM tile cols (1 bank), x2 bufs
N_WARM = 24                 # PE warmup matmuls (fill until coef lands)
N_DUMMY = 10                # trailing PE matmuls (hold k=8 through drain)

# coefT layout: [identity(128) | p-major coef blocks (16 x 128)]
CW = O_DIM * P_NUM          # 2048
COEF_COLS = 128 + CW        # 2176
HALF_A = 128 + CW // 2      # identity + blocks 0-7 -> sync ring

LAST_RESULT = None  # BassKernelResults of the most recent run (for test.py)


def build_bass(tanh_scale: float) -> bass.Bass:
    nc = bacc.Bacc("TRN2", target_bir_lowering=False)
    xt = nc.dram_tensor("xt", [P, B_CORE], mybir.dt.float8e4, kind="ExternalInput")
    coefT = nc.dram_tensor(
        "coefT", [I_DIM, COEF_COLS], mybir.dt.float16, kind="ExternalInput"
    )
    outT = nc.dram_tensor("outT", [P, B_CORE], mybir.dt.float16, kind="ExternalOutput")

    with tile.TileContext(nc) as tc:
        with (
            tc.tile_pool(name="consts", bufs=1) as consts,
            tc.tile_pool(name="xin", bufs=1) as xin_pool,
            tc.tile_pool(name="vals", bufs=4) as vals_pool,
            tc.tile_pool(name="outp", bufs=4) as out_pool,
            tc.tile_pool(name="pout", bufs=2, space="PSUM") as pout_pool,
        ):
            # Zero scratch for PE warmup, memset on the otherwise-idle DVE
            # so the PE can start ramping immediately.
            warm = consts.tile([P, P], mybir.dt.float16)
            nc.vector.memset(warm[:], 0.0)

            def tail_tile():
                return pout_pool.tile(
                    [P, TAIL_T], mybir.dt.float32, tag="o_tail", name="o_tail"
                )

            for wi in range(N_WARM):
                t = tail_tile()
                nc.tensor.matmul(t[:, :P], warm[:], warm[:], start=True, stop=True)

            coef_sb = consts.tile([P, COEF_COLS], mybir.dt.float16)
            x_tiles = [None] * len(CHUNKS)

            def load_chunk(ci, eng):
                base, wcols = CHUNKS[ci]
                x_sb = xin_pool.tile([P, wcols], mybir.dt.float8e4, tag=f"x{ci}")
                eng.dma_start(out=x_sb[:], in_=xt[:, base : base + wcols])
                x_tiles[ci] = x_sb

            # All loads ride the sync HWDGE ring (SWDGE/gpsimd loads
            # measured ~6us issue->sem vs ~2.5 on sync): chunk 0 first
            # (unblocks the tanh stream), then the coef halves (wT gates
            # the whole eviction stream), then the remaining chunks.
            load_chunk(0, nc.sync)
            nc.sync.dma_start(out=coef_sb[:, :HALF_A], in_=coefT[:, :HALF_A])
            nc.sync.dma_start(out=coef_sb[:, HALF_A:], in_=coefT[:, HALF_A:])
            for ci in range(1, len(CHUNKS)):
                load_chunk(ci, nc.sync)

            # wT[i,o] = sum_p coef via 16 identity matmuls accumulating in
            # PSUM (I.T @ block_p = block_p), then one DVE cast to f16.
            identity_h = coef_sb[:, :P]
            w_big = tail_tile()
            w_ps = w_big[:, :O_DIM]
            for k in range(P_NUM):
                nc.tensor.matmul(
                    w_ps,
                    identity_h,
                    coef_sb[:, P + k * O_DIM : P + (k + 1) * O_DIM],
                    start=(k == 0),
                    stop=(k == P_NUM - 1),
                )
            wT = consts.tile([P, O_DIM], mybir.dt.float16)
            nc.vector.tensor_copy(wT[:], w_ps)

            # --- main pipeline ---
            deferred = []  # ScalarE-drain evictions: (out_sb slice, psum)
            stores = []    # deferred stores: (ci, base, wcols, out_sb)

            for ci, (base, wcols) in enumerate(CHUNKS):
                tail = ci >= len(CHUNKS) - N_TAIL
                v_sb = vals_pool.tile([P, wcols], mybir.dt.float16, tag="v_sb")
                nc.scalar.activation(
                    v_sb[:],
                    x_tiles[ci][:],
                    mybir.ActivationFunctionType.Tanh,
                    scale=tanh_scale,
                )
                out_sb = out_pool.tile([P, wcols], mybir.dt.float16, tag="out_sb")
                # Tile widths: tail chunks all [512] on the tail ring; the
                # last main chunk ends in a [512] slice deferred to ScalarE.
                if tail:
                    widths = [TAIL_T] * (wcols // TAIL_T)
                elif ci == len(CHUNKS) - N_TAIL - 1:
                    widths, rem = [], wcols - TAIL_T
                    while rem > 0:
                        widths.append(min(MAIN_T, rem))
                        rem -= widths[-1]
                    widths.append(TAIL_T)
                else:
                    widths, rem = [], wcols
                    while rem > 0:
                        widths.append(min(MAIN_T, rem))
                        rem -= widths[-1]
                g0 = 0
                for ti, gw in enumerate(widths):
                    if tail:
                        o_ps = tail_tile()
                    else:
                        o_ps = pout_pool.tile([P, gw], mybir.dt.float32, tag="o_ps")
                        o_ps = o_ps[:]
                    for j0 in range(0, gw, NMM):
                        jw = min(NMM, gw - j0)
                        nc.tensor.matmul(
                            o_ps[:, j0 : j0 + jw],
                            wT[:],
                            v_sb[:, g0 + j0 : g0 + j0 + jw],
                            start=True,
                            stop=True,
                        )
                    # ScalarE (after its tanh stream ends) takes the tail
                    # chunks plus the last 512 of the last main chunk; DVE
                    # takes the rest.
                    if tail or (
                        ci == len(CHUNKS) - N_TAIL - 1 and ti == len(widths) - 1
                    ):
                        deferred.append((out_sb[:, g0 : g0 + gw], o_ps))
                    else:
                        nc.vector.tensor_copy(out_sb[:, g0 : g0 + gw], o_ps)
                    g0 += gw
                if ci < len(CHUNKS) - N_TAIL - 1:
                    nc.gpsimd.dma_start(
                        out=outT[:, base : base + wcols], in_=out_sb[:]
                    )
                else:
                    stores.append((base, wcols, out_sb))

            # ScalarE drain (tanh done): evict tail slices, store on the
            # idle sync ring.
            for dst, o_ps in deferred:
                nc.scalar.copy(dst, o_ps)
            for base, wcols, out_sb in stores:
                nc.sync.dma_start(out=outT[:, base : base + wcols], in_=out_sb[:])

            # Hold the PE's k=8 activity state through the store drain and
            # into the framework's semaphore-reset storm (its Tensor-side
            # resets run ~2x faster at k=8).
            for wi in range(N_DUMMY):
                t = tail_tile()
                nc.tensor.matmul(t[:, :P], warm[:], warm[:], start=True, stop=True)
    nc.finalize()
    return nc


def kernel(x, coef, tanh_range):
    global LAST_RESULT
    x = np.asarray(x, dtype=np.float32)
    coef = np.asarray(coef, dtype=np.float32)
    t = float(np.asarray(tanh_range))
    assert x.shape == (B_FULL, I_DIM), x.shape
    assert coef.shape == (O_DIM, I_DIM, P_NUM), coef.shape

    # [identity | p-major coef blocks]: block p is the [i, o] slice.
    coefT = np.empty((I_DIM, COEF_COLS), dtype=np.float16)
    coefT[:, :P] = np.eye(P, dtype=np.float16)
    coefT[:, P:] = (
        coef.transpose(1, 2, 0).astype(np.float16).reshape(I_DIM, CW)
    )
    nc = build_bass(t)
    xt_full = np.ascontiguousarray(x.T).astype(ml_dtypes.float8_e4m3)
    in_maps = [
        {"xt": np.ascontiguousarray(xt_full[:, k * B_CORE : (k + 1) * B_CORE]),
         "coefT": coefT}
        for k in range(N_CORES)
    ]
    if os.environ.get("BASS_TRACE"):
        _ensure_ntff_hook()
    res = run_bass_kernel_spmd(nc, in_maps, core_ids=list(range(N_CORES)))
    LAST_RESULT = res
    return np.concatenate(
        [r["outT"].astype(np.float32).T for r in res.results], axis=0
    )


# revision 13
# speedup vs baseline: 1.0205x; 1.0205x over previous
"""Trainium2 Bass kernel for nn_LinearLayer_45243185496808.

Computes out[b,o] = sum_i tanh(x[b,i]*t) * (sum_p coef[o,i,p]) with
B=131072, I=O=128, P_NUM=16, data-parallel over batch on 8 NeuronCores.

The kernel is elementwise-walled: ScalarE tanh (1x @1.2GHz) plus the
PSUM->SBUF fp32 eviction (1x on DVE @0.96GHz / ScalarE) are ~16-17us of
combined engine time that nothing else can absorb (GpSimd has no PSUM
port, DMA can't read PSUM, TRN2 matmul output must be fp32). Schedule
highlights, all trace-derived:

  - x ships as fp8e4 [i=128, b] (pure transpose, 1B/elem): 2 MiB/core,
    ACTIVATE reads fp8 at the same 1x rate. End-to-end rel err ~1.5e-2
    vs the 2e-2 gate (dominated by fp8 quantization of x).
  - transposed output: wT [i,o] f16 is the PE stationary; tanh values
    [i,b] f16 stream as the moving operand in N=512 blocks. PSUM holds
    out.T; host transposes the stored [128, B] f16 back.
  - HAM clock: the PE runs N=512 matmuls at 215ns only in the k=8/8
    activity state and ~630ns at k=4 (measured). Warmup matmuls on a
    zero scratch run from kernel start until the coef DMA lands so the
    PE hits k=8 before the w-reduction, and trailing dummy matmuls keep
    it there through the store drain + the framework's fixed per-engine
    semaphore-reset storm (whose Tensor-side resets run 2x faster at
    k=8 — the storm is ~half the measured tail).
  - loads alternate sync/gpsimd rings so chunk sems arrive at ~0.6us
    spacing despite the ~2.5-3us per-DMA completion latency; the coef
    (with an identity block for the w = sum_p coef reduction via 16
    accumulating identity matmuls) is the first issue on each ring.
  - PSUM = one [1536] x2 ring (6 banks) for the main chunks + one
    [512] x2 ring (2 banks) shared by warmup/w/dummies and the tail
    chunks, so the tail's matmuls never queue behind DVE's eviction
    backlog on the big ring.
  - eviction split: DVE takes ~14.3k cols; ScalarE takes the final 2048
    (c5 tail + c6 + c7) emitted after the last tanh, so both engines
    drain together; stores ride gpsimd/SWDGE except the last chunks on
    the by-then-idle sync/HWDGE ring.

HBM per core: 2 MiB x(fp8) + 0.53 MiB coef+identity + 4 MiB out(f16).
"""

import os
import sys
import types

import ml_dtypes
import numpy as np

import concourse.bass as bass
import concourse.mybir as mybir
import concourse.tile as tile
from concourse import bacc
from concourse.bass_utils import run_bass_kernel_spmd


def _ensure_ntff_hook():
    """Register the axon NTFF profile hook if the image lacks antenv.axon_hooks.

    Only needed for BASS_TRACE=1 profiling runs; harmless otherwise."""
    if "antenv.axon_hooks" in sys.modules:
        return
    try:
        from antenv.axon_hooks import get_axon_ntff_profile_hook  # noqa: F401

        return  # real module importable
    except ImportError:
        pass
    hook = None
    try:
        from trn_agent_boot.trn_boot import _ntff_profile_via_ctypes

        so_path = "/opt/axon/libaxon_pjrt.so"
        if os.path.exists(so_path):
            hook = _ntff_profile_via_ctypes(so_path)
    except Exception:
        hook = None
    mod = types.ModuleType("antenv.axon_hooks")
    mod.get_axon_ntff_profile_hook = lambda: hook
    mod.set_axon_ntff_profile_hook = lambda h: None
    sys.modules["antenv.axon_hooks"] = mod


N_CORES = 8
B_FULL = 131072
I_DIM = 128
O_DIM = 128
P_NUM = 16
P = 128                     # SBUF partitions
B_CORE = B_FULL // N_CORES  # 16384

# Load chunks of xt == tanh tiles. Fine taper up front (load-completion
# sems lag ~5us behind issue once several DMAs queue on a ring, so the
# early chunks must be small to arrive in tanh order), big middle
# (amortize the 352-cycle ACTIVATE overhead), taper down (short drain).
WIDTHS = [1024, 1536, 2048, 3072, 4096, 2048, 1536, 1024]
assert sum(WIDTHS) == B_CORE
CHUNKS = []
_b = 0
for _w in WIDTHS:
    CHUNKS.append((_b, _w))
    _b += _w
N_TAIL = 1                  # last chunk: tail PSUM ring + ScalarE eviction
N_DEFER = 2                 # ...plus the last 512 of this many prior chunks

NMM = 512                   # moving cols per matmul = one PSUM bank of f32
MAIN_T = 1536               # main PSUM tile cols (3 banks), x2 bufs
TAIL_T = 512                # tail PSUM tile cols (1 bank), x2 bufs
N_WARM = 24                 # PE warmup matmuls (fill until coef lands)
N_DUMMY = 10                # trailing PE matmuls (hold k=8 through drain)
# PE filler matmuls emitted after each chunk's real matmuls: the HAM
# activity manager drops the NC to the k=4 clock state after ~1.7us of
# PE idle, which halves matmul rate AND slows DVE/ScalarE ~10%; filler
# keeps the PE stream dense through the tanh-paced gaps.
FILLERS = {0: 3, 1: 4, 2: 5, 3: 7, 4: 9, 5: 4, 6: 3}

# coefT layout: [identity(128) | p-major coef blocks (16 x 128)]
CW = O_DIM * P_NUM          # 2048
COEF_COLS = 128 + CW        # 2176
HALF_A = 128 + CW // 2      # identity + blocks 0-7 -> sync ring

LAST_RESULT = None  # BassKernelResults of the most recent run (for test.py)


def build_bass(tanh_scale: float) -> bass.Bass:
    nc = bacc.Bacc("TRN2", target_bir_lowering=False)
    xt = nc.dram_tensor("xt", [P, B_CORE], mybir.dt.float8e4, kind="ExternalInput")
    coefT = nc.dram_tensor(
        "coefT", [I_DIM, COEF_COLS], mybir.dt.float16, kind="ExternalInput"
    )
    outT = nc.dram_tensor("outT", [P, B_CORE], mybir.dt.float16, kind="ExternalOutput")

    with tile.TileContext(nc) as tc:
        with (
            tc.tile_pool(name="consts", bufs=1) as consts,
            tc.tile_pool(name="xin", bufs=1) as xin_pool,
            tc.tile_pool(name="vals", bufs=4) as vals_pool,
            tc.tile_pool(name="outp", bufs=4) as out_pool,
            tc.tile_pool(name="pout", bufs=2, space="PSUM") as pout_pool,
        ):
            # Zero scratch for PE warmup, memset on the otherwise-idle DVE
            # so the PE can start ramping immediately.
            warm = consts.tile([P, P], mybir.dt.float16)
            nc.vector.memset(warm[:], 0.0)

            def tail_tile():
                return pout_pool.tile(
                    [P, TAIL_T], mybir.dt.float32, tag="o_tail", name="o_tail"
                )

            for wi in range(N_WARM):
                t = tail_tile()
                nc.tensor.matmul(t[:, :P], warm[:], warm[:], start=True, stop=True)

            coef_sb = consts.tile([P, COEF_COLS], mybir.dt.float16)
            x_tiles = [None] * len(CHUNKS)

            def load_chunk(ci, eng):
                base, wcols = CHUNKS[ci]
                x_sb = xin_pool.tile([P, wcols], mybir.dt.float8e4, tag=f"x{ci}")
                eng.dma_start(out=x_sb[:], in_=xt[:, base : base + wcols])
                x_tiles[ci] = x_sb

            # All loads ride the sync HWDGE ring (SWDGE/gpsimd loads
            # measured ~6us issue->sem vs ~2.5 on sync): chunk 0 first
            # (unblocks the tanh stream), then the coef halves (wT gates
            # the whole eviction stream), then the remaining chunks.
            load_chunk(0, nc.sync)
            nc.sync.dma_start(out=coef_sb[:, :HALF_A], in_=coefT[:, :HALF_A])
            nc.sync.dma_start(out=coef_sb[:, HALF_A:], in_=coefT[:, HALF_A:])
            for ci in range(1, len(CHUNKS)):
                load_chunk(ci, nc.sync)

            # wT[i,o] = sum_p coef via 16 identity matmuls accumulating in
            # PSUM (I.T @ block_p = block_p), then one DVE cast to f16.
            identity_h = coef_sb[:, :P]
            w_big = tail_tile()
            w_ps = w_big[:, :O_DIM]
            for k in range(P_NUM):
                nc.tensor.matmul(
                    w_ps,
                    identity_h,
                    coef_sb[:, P + k * O_DIM : P + (k + 1) * O_DIM],
                    start=(k == 0),
                    stop=(k == P_NUM - 1),
                )
            # wT cast on ScalarE: it lands in ScalarE's load-stall window
            # between tanh0 and tanh1 and keeps DVE's queue pure-eviction.
            wT = consts.tile([P, O_DIM], mybir.dt.float16)
            nc.scalar.copy(wT[:], w_ps)

            # --- main pipeline ---
            deferred = []  # ScalarE-drain evictions: (out_sb slice, psum)
            stores = []    # deferred stores: (ci, base, wcols, out_sb)

            for ci, (base, wcols) in enumerate(CHUNKS):
                tail = ci >= len(CHUNKS) - N_TAIL
                v_sb = vals_pool.tile([P, wcols], mybir.dt.float16, tag="v_sb")
                nc.scalar.activation(
                    v_sb[:],
                    x_tiles[ci][:],
                    mybir.ActivationFunctionType.Tanh,
                    scale=tanh_scale,
                )
                out_sb = out_pool.tile([P, wcols], mybir.dt.float16, tag="out_sb")
                # Tile widths: tail chunks all [512] on the tail ring; the
                # last N_DEFER main chunks end in a [512] slice deferred to
                # ScalarE.
                split_last = (
                    len(CHUNKS) - N_TAIL - N_DEFER <= ci < len(CHUNKS) - N_TAIL
                )
                if tail:
                    widths = [TAIL_T] * (wcols // TAIL_T)
                else:
                    widths, rem = [], wcols - (TAIL_T if split_last else 0)
                    while rem > 0:
                        widths.append(min(MAIN_T, rem))
                        rem -= widths[-1]
                    if split_last:
                        widths.append(TAIL_T)
                g0 = 0
                for ti, gw in enumerate(widths):
                    if tail:
                        o_ps = tail_tile()
                    else:
                        o_ps = pout_pool.tile([P, gw], mybir.dt.float32, tag="o_ps")
                        o_ps = o_ps[:]
                    for j0 in range(0, gw, NMM):
                        jw = min(NMM, gw - j0)
                        nc.tensor.matmul(
                            o_ps[:, j0 : j0 + jw],
                            wT[:],
                            v_sb[:, g0 + j0 : g0 + j0 + jw],
                            start=True,
                            stop=True,
                        )
                    # ScalarE (after its tanh stream ends) takes the tail
                    # chunk plus the last 512 of the N_DEFER chunks before
                    # it; DVE takes the rest.
                    if tail or (split_last and ti == len(widths) - 1):
                        deferred.append((out_sb[:, g0 : g0 + gw], o_ps))
                    else:
                        nc.vector.tensor_copy(out_sb[:, g0 : g0 + gw], o_ps)
                    g0 += gw
                # Filler matmuls keep the PE stream dense (HAM k=8).
                for _ in range(FILLERS.get(ci, 0)):
                    t = tail_tile()
                    nc.tensor.matmul(
                        t[:, :P], warm[:], warm[:], start=True, stop=True
                    )
                if ci < len(CHUNKS) - N_TAIL - N_DEFER:
                    nc.gpsimd.dma_start(
                        out=outT[:, base : base + wcols], in_=out_sb[:]
                    )
                else:
                    # Chunks with a deferred eviction store only after the
                    # ScalarE drain completes their out tile.
                    stores.append((base, wcols, out_sb))

            # ScalarE drain (tanh done): evict tail slices, store on the
            # idle sync ring.
            for dst, o_ps in deferred:
                nc.scalar.copy(dst, o_ps)
            for base, wcols, out_sb in stores:
                nc.sync.dma_start(out=outT[:, base : base + wcols], in_=out_sb[:])

            # Hold the PE's k=8 activity state through the store drain and
            # into the framework's semaphore-reset storm (its Tensor-side
            # resets run ~2x faster at k=8).
            for wi in range(N_DUMMY):
                t = tail_tile()
                nc.tensor.matmul(t[:, :P], warm[:], warm[:], start=True, stop=True)
    nc.finalize()
    return nc


def kernel(x, coef, tanh_range):
    global LAST_RESULT
    x = np.asarray(x, dtype=np.float32)
    coef = np.asarray(coef, dtype=np.float32)
    t = float(np.asarray(tanh_range))
    assert x.shape == (B_FULL, I_DIM), x.shape
    assert coef.shape == (O_DIM, I_DIM, P_NUM), coef.shape

    # [identity | p-major coef blocks]: block p is the [i, o] slice.
    coefT = np.empty((I_DIM, COEF_COLS), dtype=np.float16)
    coefT[:, :P] = np.eye(P, dtype=np.float16)
    coefT[:, P:] = (
        coef.transpose(1, 2, 0).astype(np.float16).reshape(I_DIM, CW)
    )
    nc = build_bass(t)
    xt_full = np.ascontiguousarray(x.T).astype(ml_dtypes.float8_e4m3)
    in_maps = [
        {"xt": np.ascontiguousarray(xt_full[:, k * B_CORE : (k + 1) * B_CORE]),
         "coefT": coefT}
        for k in range(N_CORES)
    ]
    if os.environ.get("BASS_TRACE"):
        _ensure_ntff_hook()
    res = run_bass_kernel_spmd(nc, in_maps, core_ids=list(range(N_CORES)))
    LAST_RESULT = res
    return np.concatenate(
        [r["outT"].astype(np.float32).T for r in res.results], axis=0
    )
